# revision 11
# baseline (speedup 1.0000x reference)
"""HeteroGCNConv Trainium2 kernel: 8-core dst-sharded gather/one-hot-matmul.

Algorithm (per edge type): out = D_t^-1/2 A D_s^-1/2 x W, computed as
S^T accumulated in PSUM via per-chunk one-hot matmuls over gathered source
rows, then a 128x128 weight matmul, relu, and a row-scatter.

Sharding: destination nodes split into 8 contiguous ranges (one per core);
host groups each core's edges by dst into tiles of <=128 nodes x K chunks
of 128 edges; per-edge norm 1/sqrt(deg_s*deg_t) is baked into the one-hot.
out_a tiles fuse the ba+aa branches in one PSUM accumulation.
"""
import sys
sys.path.insert(0, "/opt/trn_rl_repo")
import numpy as np

P = 128
OOB = 1 << 20

# full problem dims (hardcoded per contract)
N_A, N_B = 100_000, 50_000
N_CORES = 8

_BUILD_CACHE = {}


# ---------------------------------------------------------------- host packing
def _pack_pipeline(srcs, dsts, deg_s, deg_t, r0, r1, K_list, n_core_nodes):
    """Pack one core's edges (for one or two edge types sharing node tiles).

    srcs/dsts/deg_s/deg_t/K_list: lists (one entry per edge type).
    Returns per-type (srcT, dstT, nrmT) with T tiles, plus rowT and T.
    Edges must be pre-filtered to dst in [r0, r1) and sorted by dst.
    """
    ntype = len(srcs)
    # per-node degree within this core, per type
    local_deg = [np.bincount(d - r0, minlength=n_core_nodes) for d in dsts]
    # greedy: close tile when >128 nodes or any type exceeds K*128 edges
    caps = [k * P for k in K_list]
    tile_of_node = np.empty(n_core_nodes, np.int32)
    slot_of_node = np.empty(n_core_nodes, np.int32)
    t = 0
    cnt_nodes = 0
    cnt_edges = [0] * ntype
    for n in range(n_core_nodes):
        degs = [int(local_deg[i][n]) for i in range(ntype)]
        if cnt_nodes + 1 > P or any(cnt_edges[i] + degs[i] > caps[i]
                                    for i in range(ntype)):
            t += 1
            cnt_nodes = 0
            cnt_edges = [0] * ntype
        tile_of_node[n] = t
        slot_of_node[n] = cnt_nodes
        cnt_nodes += 1
        for i in range(ntype):
            cnt_edges[i] += degs[i]
    T = t + 1

    rowT = np.full((P, T), OOB, np.int32)
    rowT[slot_of_node, tile_of_node] = np.arange(n_core_nodes, dtype=np.int32)

    out = []
    for i in range(ntype):
        K = K_list[i]
        src, dst = srcs[i], dsts[i]
        ld = local_deg[i]
        # edges sorted by dst; tile edge ranges via per-tile degree sums
        tile_deg = np.bincount(tile_of_node, weights=ld, minlength=T).astype(np.int64)
        tile_start = np.concatenate([[0], np.cumsum(tile_deg)])[:-1]
        etile = tile_of_node[dst - r0]
        pos_in_tile = np.arange(len(dst)) - tile_start[etile]
        k = (pos_in_tile // P).astype(np.int64)
        p = (pos_in_tile % P).astype(np.int64)
        col = etile * K + k
        assert k.max(initial=0) < K, f"K={K} too small (max {k.max()+1})"
        srcT = np.zeros((P, T * K), np.int32)
        dstT = np.zeros((P, T * K), np.float32)
        nrmT = np.zeros((P, T * K), np.float32)
        srcT[p, col] = src
        dstT[p, col] = slot_of_node[dst - r0]
        nrmT[p, col] = 1.0 / np.sqrt(deg_s[i][src] * deg_t[i][dst])
        out.append((srcT, dstT, nrmT))
    return out, rowT, T


def _pad_tiles(packed, rowT, T, T_max, K_list):
    """Pad a core's packing out to T_max tiles (all-padding tiles)."""
    if T == T_max:
        return packed, rowT
    out = []
    for (srcT, dstT, nrmT), K in zip(packed, K_list):
        pad = np.zeros((P, (T_max - T) * K), srcT.dtype)
        srcT = np.concatenate([srcT, pad.astype(np.int32)], axis=1)
        dstT = np.concatenate([dstT, pad.astype(np.float32)], axis=1)
        nrmT = np.concatenate([nrmT, pad.astype(np.float32)], axis=1)
        out.append((srcT, dstT, nrmT))
    rowT = np.concatenate([rowT, np.full((P, T_max - T), OOB, np.int32)], axis=1)
    return out, rowT


def host_prepare(x_a, x_b, src_ab, dst_ab, src_ba, dst_ba, src_aa, dst_aa,
                 n_a, n_b, K_ab, K_ba, K_aa):
    """Shard + sort + pack all edges. Returns per-core aux arrays and (T1, T2)."""
    deg = {}
    deg["s_ab"] = np.bincount(src_ab, minlength=n_a).astype(np.float64)
    deg["t_ab"] = np.bincount(dst_ab, minlength=n_b).astype(np.float64)
    deg["s_ba"] = np.bincount(src_ba, minlength=n_b).astype(np.float64)
    deg["t_ba"] = np.bincount(dst_ba, minlength=n_a).astype(np.float64)
    deg["s_aa"] = np.bincount(src_aa, minlength=n_a).astype(np.float64)
    deg["t_aa"] = np.bincount(dst_aa, minlength=n_a).astype(np.float64)

    SA, SB = n_a // N_CORES, n_b // N_CORES

    def sort_split(src, dst, S):
        o = np.argsort(dst, kind="stable")
        src, dst = src[o], dst[o]
        bounds = np.searchsorted(dst, np.arange(0, S * (N_CORES + 1), S))
        return src, dst, bounds

    s_ab, d_ab, b_ab = sort_split(src_ab, dst_ab, SB)
    s_ba, d_ba, b_ba = sort_split(src_ba, dst_ba, SA)
    s_aa, d_aa, b_aa = sort_split(src_aa, dst_aa, SA)

    cores = []
    for c in range(N_CORES):
        # pipeline 1: ab -> out_b
        sl = slice(b_ab[c], b_ab[c + 1])
        p1, rowB, T1 = _pack_pipeline(
            [s_ab[sl]], [d_ab[sl]], [deg["s_ab"]], [deg["t_ab"]],
            c * SB, (c + 1) * SB, [K_ab], SB)
        # pipeline 2+3: ba,aa -> out_a (shared tiles)
        sl2, sl3 = slice(b_ba[c], b_ba[c + 1]), slice(b_aa[c], b_aa[c + 1])
        p23, rowA, T2 = _pack_pipeline(
            [s_ba[sl2], s_aa[sl3]], [d_ba[sl2], d_aa[sl3]],
            [deg["s_ba"], deg["s_aa"]], [deg["t_ba"], deg["t_aa"]],
            c * SA, (c + 1) * SA, [K_ba, K_aa], SA)
        cores.append([p1, rowB, T1, p23, rowA, T2])

    T1_max = max(c[2] for c in cores)
    T2_max = max(c[5] for c in cores)
    in_maps = []
    for c in range(N_CORES):
        p1, rowB, T1, p23, rowA, T2 = cores[c]
        p1, rowB = _pad_tiles(p1, rowB, T1, T1_max, [K_ab])
        p23, rowA = _pad_tiles(p23, rowA, T2, T2_max, [K_ba, K_aa])
        m = dict(
            x_a=x_a, x_b=x_b,
            src_ab=p1[0][0], dst_ab=p1[0][1], nrm_ab=p1[0][2], row_b=rowB,
            src_ba=p23[0][0], dst_ba=p23[0][1], nrm_ba=p23[0][2],
            src_aa=p23[1][0], dst_aa=p23[1][1], nrm_aa=p23[1][2], row_a=rowA,
        )
        in_maps.append(m)
    return in_maps, T1_max, T2_max


# ---------------------------------------------------------------- device build
def build_nc(n_a, n_b, T1, K_ab, T2, K_ba, K_aa, SA, SB):
    from concourse import bass, bacc, mybir, tile

    f32 = mybir.dt.float32
    i32 = mybir.dt.int32
    nc = bacc.Bacc(None, target_bir_lowering=False, debug=False)

    x_a = nc.dram_tensor("x_a", [n_a, P], f32, kind="ExternalInput")
    x_b = nc.dram_tensor("x_b", [n_b, P], f32, kind="ExternalInput")
    W_ab = nc.dram_tensor("W_ab", [P, P], f32, kind="ExternalInput")
    W_ba = nc.dram_tensor("W_ba", [P, P], f32, kind="ExternalInput")
    W_aa = nc.dram_tensor("W_aa", [P, P], f32, kind="ExternalInput")
    aux = {}
    for nm, C in (("src_ab", T1 * K_ab), ("dst_ab", T1 * K_ab), ("nrm_ab", T1 * K_ab),
                  ("src_ba", T2 * K_ba), ("dst_ba", T2 * K_ba), ("nrm_ba", T2 * K_ba),
                  ("src_aa", T2 * K_aa), ("dst_aa", T2 * K_aa), ("nrm_aa", T2 * K_aa)):
        dt = i32 if nm.startswith("src") else f32
        aux[nm] = nc.dram_tensor(nm, [P, C], dt, kind="ExternalInput")
    row_b = nc.dram_tensor("row_b", [P, T1], i32, kind="ExternalInput")
    row_a = nc.dram_tensor("row_a", [P, T2], i32, kind="ExternalInput")
    out_a = nc.dram_tensor("out_a_part", [SA, P], f32, kind="ExternalOutput")
    out_b = nc.dram_tensor("out_b_part", [SB, P], f32, kind="ExternalOutput")

    Relu = mybir.ActivationFunctionType.Relu

    with tile.TileContext(nc) as tc:
        with (
            tc.tile_pool(name="const", bufs=1) as cpool,
            tc.tile_pool(name="g", bufs=12) as gpool,
            tc.tile_pool(name="oh", bufs=12) as ohpool,
            tc.tile_pool(name="stg", bufs=6) as stpool,
            tc.tile_pool(name="psS", bufs=3, space="PSUM") as psS,
            tc.tile_pool(name="psO", bufs=2, space="PSUM") as psO,
        ):
            iota_i = cpool.tile([P, P], i32)
            nc.gpsimd.iota(iota_i[:], pattern=[[1, P]], base=0, channel_multiplier=0)
            iota_f = cpool.tile([P, P], f32)
            nc.vector.tensor_copy(iota_f[:], iota_i[:])
            w_sb = {}
            for nm, t in (("W_ab", W_ab), ("W_ba", W_ba), ("W_aa", W_aa)):
                w_sb[nm] = cpool.tile([P, P], f32, name="w_" + nm, tag="w_" + nm)
                nc.sync.dma_start(out=w_sb[nm][:], in_=t[:])
            sb = {}
            for nm, t in aux.items():
                sb[nm] = cpool.tile([P, t.shape[1]], t.dtype, name="sb_" + nm, tag="sb_" + nm)
                nc.sync.dma_start(out=sb[nm][:], in_=t[:])
            row_b_sb = cpool.tile([P, T1], i32)
            nc.sync.dma_start(out=row_b_sb[:], in_=row_b[:])
            row_a_sb = cpool.tile([P, T2], i32)
            nc.sync.dma_start(out=row_a_sb[:], in_=row_a[:])

            def accum_half(x_tab, pre, K, t, tag):
                """Gather+one-hot+matmul K chunks; returns psum_S tile [f, slot]."""
                psum_S = psS.tile([P, P], f32, tag=("S12" if tag in ("1", "2") else "S3"))
                for k in range(K):
                    c = t * K + k
                    g = gpool.tile([P, P], f32, tag="g" + tag)
                    nc.gpsimd.indirect_dma_start(
                        out=g[:], out_offset=None, in_=x_tab[:],
                        in_offset=bass.IndirectOffsetOnAxis(
                            ap=sb["src_" + pre][:, c:c + 1], axis=0))
                    oh = ohpool.tile([P, P], f32, tag="oh" + tag)
                    nc.vector.tensor_scalar(
                        out=oh[:], in0=iota_f[:],
                        scalar1=sb["dst_" + pre][:, c:c + 1],
                        scalar2=sb["nrm_" + pre][:, c:c + 1],
                        op0=mybir.AluOpType.is_equal,
                        op1=mybir.AluOpType.mult)
                    nc.tensor.matmul(out=psum_S[:], lhsT=g[:], rhs=oh[:],
                                     start=(k == 0), stop=(k == K - 1))
                return psum_S

            # pipeline 1: ab -> out_b
            for t in range(T1):
                pS = accum_half(x_a, "ab", K_ab, t, "1")
                st = stpool.tile([P, P], f32, tag="st1")
                nc.scalar.activation(st[:], pS[:],
                                     mybir.ActivationFunctionType.Copy)
                pO = psO.tile([P, P], f32, tag="O")
                nc.tensor.matmul(out=pO[:], lhsT=st[:], rhs=w_sb["W_ab"][:],
                                 start=True, stop=True)
                ot = stpool.tile([P, P], f32, tag="ot1")
                nc.scalar.activation(ot[:], pO[:], Relu)
                nc.gpsimd.indirect_dma_start(
                    out=out_b[:],
                    out_offset=bass.IndirectOffsetOnAxis(
                        ap=row_b_sb[:, t:t + 1], axis=0),
                    in_=ot[:], in_offset=None,
                    bounds_check=SB - 1, oob_is_err=False)

            # pipeline 2+3: ba+aa -> out_a
            for t in range(T2):
                pS1 = accum_half(x_b, "ba", K_ba, t, "2")
                pS2 = accum_half(x_a, "aa", K_aa, t, "3")
                st1 = stpool.tile([P, P], f32, tag="st2")
                nc.scalar.activation(st1[:], pS1[:],
                                     mybir.ActivationFunctionType.Copy)
                st2 = stpool.tile([P, P], f32, tag="st3")
                nc.scalar.activation(st2[:], pS2[:],
                                     mybir.ActivationFunctionType.Copy)
                pO = psO.tile([P, P], f32, tag="O")
                nc.tensor.matmul(out=pO[:], lhsT=st1[:], rhs=w_sb["W_ba"][:],
                                 start=True, stop=False)
                nc.tensor.matmul(out=pO[:], lhsT=st2[:], rhs=w_sb["W_aa"][:],
                                 start=False, stop=True)
                ot = stpool.tile([P, P], f32, tag="ot2")
                nc.scalar.activation(ot[:], pO[:], Relu, scale=0.5)
                nc.gpsimd.indirect_dma_start(
                    out=out_a[:],
                    out_offset=bass.IndirectOffsetOnAxis(
                        ap=row_a_sb[:, t:t + 1], axis=0),
                    in_=ot[:], in_offset=None,
                    bounds_check=SA - 1, oob_is_err=False)
    nc.compile()
    return nc


# ---------------------------------------------------------------- entry point
def kernel(x_a, x_b, W_ab, W_ba, W_aa,
           src_ab, dst_ab, src_ba, dst_ba, src_aa, dst_aa,
           _n_a=None, _n_b=None):
    from concourse.bass_utils import run_bass_kernel_spmd

    n_a = _n_a or x_a.shape[0]
    n_b = _n_b or x_b.shape[0]
    K_ab, K_ba, K_aa = 22, 12, 12

    x_a = np.ascontiguousarray(np.asarray(x_a, dtype=np.float32))
    x_b = np.ascontiguousarray(np.asarray(x_b, dtype=np.float32))
    in_maps, T1, T2 = host_prepare(
        x_a, x_b,
        np.asarray(src_ab), np.asarray(dst_ab),
        np.asarray(src_ba), np.asarray(dst_ba),
        np.asarray(src_aa), np.asarray(dst_aa),
        n_a, n_b, K_ab, K_ba, K_aa)
    for m in in_maps:
        m["W_ab"] = np.asarray(W_ab, dtype=np.float32)
        m["W_ba"] = np.asarray(W_ba, dtype=np.float32)
        m["W_aa"] = np.asarray(W_aa, dtype=np.float32)

    SA, SB = n_a // N_CORES, n_b // N_CORES
    key = (n_a, n_b, T1, T2)
    if key not in _BUILD_CACHE:
        _BUILD_CACHE[key] = build_nc(n_a, n_b, T1, K_ab, T2, K_ba, K_aa, SA, SB)
    nc = _BUILD_CACHE[key]
    import time as _time
    _t0 = _time.perf_counter()
    res = run_bass_kernel_spmd(nc, in_maps, list(range(N_CORES)))
    global LAST_EXEC_WALL_NS
    LAST_EXEC_WALL_NS = int((_time.perf_counter() - _t0) * 1e9)
    out_a = np.concatenate([res.results[c]["out_a_part"] for c in range(N_CORES)])
    out_b = np.concatenate([res.results[c]["out_b_part"] for c in range(N_CORES)])
    return out_a, out_b


# revision 12
# speedup vs baseline: 1.1002x; 1.1002x over previous
"""HeteroGCNConv Trainium2 kernel: 8-core dst-sharded gather/one-hot-matmul.

Algorithm (per edge type): out = D_t^-1/2 A D_s^-1/2 x W, computed as
S^T accumulated in PSUM via per-chunk one-hot matmuls over gathered source
rows, then a 128x128 weight matmul, relu, and a row-scatter.

Sharding: destination nodes split into 8 contiguous ranges (one per core);
host groups each core's edges by dst into tiles of <=128 nodes x K chunks
of 128 edges; per-edge norm 1/sqrt(deg_s*deg_t) is baked into the one-hot.
out_a tiles fuse the ba+aa branches in one PSUM accumulation.
"""
import sys
sys.path.insert(0, "/opt/trn_rl_repo")
import numpy as np

P = 128
OOB = 1 << 20

# full problem dims (hardcoded per contract)
N_A, N_B = 100_000, 50_000
N_CORES = 8

_BUILD_CACHE = {}


# ---------------------------------------------------------------- host packing
def _pack_pipeline(srcs, dsts, deg_s, deg_t, r0, r1, K_list, n_core_nodes):
    """Pack one core's edges (for one or two edge types sharing node tiles).

    srcs/dsts/deg_s/deg_t/K_list: lists (one entry per edge type).
    Returns per-type (srcT, dstT, nrmT) with T tiles, plus rowT and T.
    Edges must be pre-filtered to dst in [r0, r1) and sorted by dst.
    """
    ntype = len(srcs)
    # per-node degree within this core, per type
    local_deg = [np.bincount(d - r0, minlength=n_core_nodes) for d in dsts]
    # greedy: close tile when >128 nodes or any type exceeds K*128 edges
    caps = [k * P for k in K_list]
    tile_of_node = np.empty(n_core_nodes, np.int32)
    slot_of_node = np.empty(n_core_nodes, np.int32)
    t = 0
    cnt_nodes = 0
    cnt_edges = [0] * ntype
    for n in range(n_core_nodes):
        degs = [int(local_deg[i][n]) for i in range(ntype)]
        if cnt_nodes + 1 > P or any(cnt_edges[i] + degs[i] > caps[i]
                                    for i in range(ntype)):
            t += 1
            cnt_nodes = 0
            cnt_edges = [0] * ntype
        tile_of_node[n] = t
        slot_of_node[n] = cnt_nodes
        cnt_nodes += 1
        for i in range(ntype):
            cnt_edges[i] += degs[i]
    T = t + 1

    rowT = np.full((P, T), OOB, np.int32)
    rowT[slot_of_node, tile_of_node] = np.arange(n_core_nodes, dtype=np.int32)

    out = []
    for i in range(ntype):
        K = K_list[i]
        src, dst = srcs[i], dsts[i]
        ld = local_deg[i]
        # edges sorted by dst; tile edge ranges via per-tile degree sums
        tile_deg = np.bincount(tile_of_node, weights=ld, minlength=T).astype(np.int64)
        tile_start = np.concatenate([[0], np.cumsum(tile_deg)])[:-1]
        etile = tile_of_node[dst - r0]
        pos_in_tile = np.arange(len(dst)) - tile_start[etile]
        k = (pos_in_tile // P).astype(np.int64)
        p = (pos_in_tile % P).astype(np.int64)
        col = etile * K + k
        assert k.max(initial=0) < K, f"K={K} too small (max {k.max()+1})"
        srcT = np.zeros((P, T * K), np.int32)
        dstT = np.zeros((P, T * K), np.float32)
        nrmT = np.zeros((P, T * K), np.float32)
        srcT[p, col] = src
        dstT[p, col] = slot_of_node[dst - r0]
        nrmT[p, col] = 1.0 / np.sqrt(deg_s[i][src] * deg_t[i][dst])
        out.append((srcT, dstT, nrmT))
    return out, rowT, T


def _pad_tiles(packed, rowT, T, T_max, K_list):
    """Pad a core's packing out to T_max tiles (all-padding tiles)."""
    if T == T_max:
        return packed, rowT
    out = []
    for (srcT, dstT, nrmT), K in zip(packed, K_list):
        pad = np.zeros((P, (T_max - T) * K), srcT.dtype)
        srcT = np.concatenate([srcT, pad.astype(np.int32)], axis=1)
        dstT = np.concatenate([dstT, pad.astype(np.float32)], axis=1)
        nrmT = np.concatenate([nrmT, pad.astype(np.float32)], axis=1)
        out.append((srcT, dstT, nrmT))
    rowT = np.concatenate([rowT, np.full((P, T_max - T), OOB, np.int32)], axis=1)
    return out, rowT


def host_prepare(x_a, x_b, src_ab, dst_ab, src_ba, dst_ba, src_aa, dst_aa,
                 n_a, n_b, K_ab, K_ba, K_aa):
    """Shard + sort + pack all edges. Returns per-core aux arrays and (T1, T2)."""
    deg = {}
    deg["s_ab"] = np.bincount(src_ab, minlength=n_a).astype(np.float64)
    deg["t_ab"] = np.bincount(dst_ab, minlength=n_b).astype(np.float64)
    deg["s_ba"] = np.bincount(src_ba, minlength=n_b).astype(np.float64)
    deg["t_ba"] = np.bincount(dst_ba, minlength=n_a).astype(np.float64)
    deg["s_aa"] = np.bincount(src_aa, minlength=n_a).astype(np.float64)
    deg["t_aa"] = np.bincount(dst_aa, minlength=n_a).astype(np.float64)

    SA, SB = n_a // N_CORES, n_b // N_CORES

    def sort_split(src, dst, S):
        o = np.argsort(dst, kind="stable")
        src, dst = src[o], dst[o]
        bounds = np.searchsorted(dst, np.arange(0, S * (N_CORES + 1), S))
        return src, dst, bounds

    s_ab, d_ab, b_ab = sort_split(src_ab, dst_ab, SB)
    s_ba, d_ba, b_ba = sort_split(src_ba, dst_ba, SA)
    s_aa, d_aa, b_aa = sort_split(src_aa, dst_aa, SA)

    cores = []
    for c in range(N_CORES):
        # pipeline 1: ab -> out_b
        sl = slice(b_ab[c], b_ab[c + 1])
        p1, rowB, T1 = _pack_pipeline(
            [s_ab[sl]], [d_ab[sl]], [deg["s_ab"]], [deg["t_ab"]],
            c * SB, (c + 1) * SB, [K_ab], SB)
        # pipeline 2+3: ba,aa -> out_a (shared tiles)
        sl2, sl3 = slice(b_ba[c], b_ba[c + 1]), slice(b_aa[c], b_aa[c + 1])
        p23, rowA, T2 = _pack_pipeline(
            [s_ba[sl2], s_aa[sl3]], [d_ba[sl2], d_aa[sl3]],
            [deg["s_ba"], deg["s_aa"]], [deg["t_ba"], deg["t_aa"]],
            c * SA, (c + 1) * SA, [K_ba, K_aa], SA)
        cores.append([p1, rowB, T1, p23, rowA, T2])

    T1_max = max(c[2] for c in cores)
    T2_max = max(c[5] for c in cores)
    in_maps = []
    for c in range(N_CORES):
        p1, rowB, T1, p23, rowA, T2 = cores[c]
        p1, rowB = _pad_tiles(p1, rowB, T1, T1_max, [K_ab])
        p23, rowA = _pad_tiles(p23, rowA, T2, T2_max, [K_ba, K_aa])
        m = dict(
            x_a=x_a, x_b=x_b,
            src_ab=p1[0][0], dst_ab=p1[0][1], nrm_ab=p1[0][2], row_b=rowB,
            src_ba=p23[0][0], dst_ba=p23[0][1], nrm_ba=p23[0][2],
            src_aa=p23[1][0], dst_aa=p23[1][1], nrm_aa=p23[1][2], row_a=rowA,
        )
        in_maps.append(m)
    return in_maps, T1_max, T2_max


# ---------------------------------------------------------------- device build
def build_nc(n_a, n_b, T1, K_ab, T2, K_ba, K_aa, SA, SB):
    from concourse import bass, bacc, mybir, tile

    f32 = mybir.dt.float32
    i32 = mybir.dt.int32
    nc = bacc.Bacc(None, target_bir_lowering=False, debug=False, num_swdge_queues=4)

    x_a = nc.dram_tensor("x_a", [n_a, P], f32, kind="ExternalInput")
    x_b = nc.dram_tensor("x_b", [n_b, P], f32, kind="ExternalInput")
    W_ab = nc.dram_tensor("W_ab", [P, P], f32, kind="ExternalInput")
    W_ba = nc.dram_tensor("W_ba", [P, P], f32, kind="ExternalInput")
    W_aa = nc.dram_tensor("W_aa", [P, P], f32, kind="ExternalInput")
    aux = {}
    for nm, C in (("src_ab", T1 * K_ab), ("dst_ab", T1 * K_ab), ("nrm_ab", T1 * K_ab),
                  ("src_ba", T2 * K_ba), ("dst_ba", T2 * K_ba), ("nrm_ba", T2 * K_ba),
                  ("src_aa", T2 * K_aa), ("dst_aa", T2 * K_aa), ("nrm_aa", T2 * K_aa)):
        dt = i32 if nm.startswith("src") else f32
        aux[nm] = nc.dram_tensor(nm, [P, C], dt, kind="ExternalInput")
    row_b = nc.dram_tensor("row_b", [P, T1], i32, kind="ExternalInput")
    row_a = nc.dram_tensor("row_a", [P, T2], i32, kind="ExternalInput")
    out_a = nc.dram_tensor("out_a_part", [SA, P], f32, kind="ExternalOutput")
    out_b = nc.dram_tensor("out_b_part", [SB, P], f32, kind="ExternalOutput")

    Relu = mybir.ActivationFunctionType.Relu

    with tile.TileContext(nc) as tc:
        with (
            tc.tile_pool(name="const", bufs=1) as cpool,
            tc.tile_pool(name="g", bufs=12) as gpool,
            tc.tile_pool(name="oh", bufs=12) as ohpool,
            tc.tile_pool(name="stg", bufs=6) as stpool,
            tc.tile_pool(name="psS", bufs=3, space="PSUM") as psS,
            tc.tile_pool(name="psO", bufs=2, space="PSUM") as psO,
        ):
            iota_i = cpool.tile([P, P], i32)
            nc.gpsimd.iota(iota_i[:], pattern=[[1, P]], base=0, channel_multiplier=0)
            iota_f = cpool.tile([P, P], f32)
            nc.vector.tensor_copy(iota_f[:], iota_i[:])
            w_sb = {}
            for nm, t in (("W_ab", W_ab), ("W_ba", W_ba), ("W_aa", W_aa)):
                w_sb[nm] = cpool.tile([P, P], f32, name="w_" + nm, tag="w_" + nm)
                nc.sync.dma_start(out=w_sb[nm][:], in_=t[:])
            sb = {}
            for nm, t in aux.items():
                sb[nm] = cpool.tile([P, t.shape[1]], t.dtype, name="sb_" + nm, tag="sb_" + nm)
                nc.sync.dma_start(out=sb[nm][:], in_=t[:])
            row_b_sb = cpool.tile([P, T1], i32)
            nc.sync.dma_start(out=row_b_sb[:], in_=row_b[:])
            row_a_sb = cpool.tile([P, T2], i32)
            nc.sync.dma_start(out=row_a_sb[:], in_=row_a[:])

            def accum_half(x_tab, pre, K, t, tag):
                """Gather+one-hot+matmul K chunks; returns psum_S tile [f, slot]."""
                psum_S = psS.tile([P, P], f32, tag=("S12" if tag in ("1", "2") else "S3"))
                for k in range(K):
                    c = t * K + k
                    g = gpool.tile([P, P], f32, tag="g" + tag)
                    nc.gpsimd.indirect_dma_start(
                        out=g[:], out_offset=None, in_=x_tab[:],
                        in_offset=bass.IndirectOffsetOnAxis(
                            ap=sb["src_" + pre][:, c:c + 1], axis=0))
                    oh = ohpool.tile([P, P], f32, tag="oh" + tag)
                    nc.vector.tensor_scalar(
                        out=oh[:], in0=iota_f[:],
                        scalar1=sb["dst_" + pre][:, c:c + 1],
                        scalar2=sb["nrm_" + pre][:, c:c + 1],
                        op0=mybir.AluOpType.is_equal,
                        op1=mybir.AluOpType.mult)
                    nc.tensor.matmul(out=psum_S[:], lhsT=g[:], rhs=oh[:],
                                     start=(k == 0), stop=(k == K - 1))
                return psum_S

            # pipeline 1: ab -> out_b
            for t in range(T1):
                pS = accum_half(x_a, "ab", K_ab, t, "1")
                st = stpool.tile([P, P], f32, tag="st1")
                nc.scalar.activation(st[:], pS[:],
                                     mybir.ActivationFunctionType.Copy)
                pO = psO.tile([P, P], f32, tag="O")
                nc.tensor.matmul(out=pO[:], lhsT=st[:], rhs=w_sb["W_ab"][:],
                                 start=True, stop=True)
                ot = stpool.tile([P, P], f32, tag="ot1")
                nc.scalar.activation(ot[:], pO[:], Relu)
                nc.gpsimd.indirect_dma_start(
                    out=out_b[:],
                    out_offset=bass.IndirectOffsetOnAxis(
                        ap=row_b_sb[:, t:t + 1], axis=0),
                    in_=ot[:], in_offset=None,
                    bounds_check=SB - 1, oob_is_err=False)

            # pipeline 2+3: ba+aa -> out_a
            for t in range(T2):
                pS1 = accum_half(x_b, "ba", K_ba, t, "2")
                pS2 = accum_half(x_a, "aa", K_aa, t, "3")
                st1 = stpool.tile([P, P], f32, tag="st2")
                nc.scalar.activation(st1[:], pS1[:],
                                     mybir.ActivationFunctionType.Copy)
                st2 = stpool.tile([P, P], f32, tag="st3")
                nc.scalar.activation(st2[:], pS2[:],
                                     mybir.ActivationFunctionType.Copy)
                pO = psO.tile([P, P], f32, tag="O")
                nc.tensor.matmul(out=pO[:], lhsT=st1[:], rhs=w_sb["W_ba"][:],
                                 start=True, stop=False)
                nc.tensor.matmul(out=pO[:], lhsT=st2[:], rhs=w_sb["W_aa"][:],
                                 start=False, stop=True)
                ot = stpool.tile([P, P], f32, tag="ot2")
                nc.scalar.activation(ot[:], pO[:], Relu, scale=0.5)
                nc.gpsimd.indirect_dma_start(
                    out=out_a[:],
                    out_offset=bass.IndirectOffsetOnAxis(
                        ap=row_a_sb[:, t:t + 1], axis=0),
                    in_=ot[:], in_offset=None,
                    bounds_check=SA - 1, oob_is_err=False)
    nc.compile()
    return nc


# ---------------------------------------------------------------- entry point
def kernel(x_a, x_b, W_ab, W_ba, W_aa,
           src_ab, dst_ab, src_ba, dst_ba, src_aa, dst_aa,
           _n_a=None, _n_b=None):
    from concourse.bass_utils import run_bass_kernel_spmd

    n_a = _n_a or x_a.shape[0]
    n_b = _n_b or x_b.shape[0]
    K_ab, K_ba, K_aa = 22, 12, 12

    x_a = np.ascontiguousarray(np.asarray(x_a, dtype=np.float32))
    x_b = np.ascontiguousarray(np.asarray(x_b, dtype=np.float32))
    in_maps, T1, T2 = host_prepare(
        x_a, x_b,
        np.asarray(src_ab), np.asarray(dst_ab),
        np.asarray(src_ba), np.asarray(dst_ba),
        np.asarray(src_aa), np.asarray(dst_aa),
        n_a, n_b, K_ab, K_ba, K_aa)
    for m in in_maps:
        m["W_ab"] = np.asarray(W_ab, dtype=np.float32)
        m["W_ba"] = np.asarray(W_ba, dtype=np.float32)
        m["W_aa"] = np.asarray(W_aa, dtype=np.float32)

    SA, SB = n_a // N_CORES, n_b // N_CORES
    key = (n_a, n_b, T1, T2)
    if key not in _BUILD_CACHE:
        _BUILD_CACHE[key] = build_nc(n_a, n_b, T1, K_ab, T2, K_ba, K_aa, SA, SB)
    nc = _BUILD_CACHE[key]
    import time as _time
    _t0 = _time.perf_counter()
    res = run_bass_kernel_spmd(nc, in_maps, list(range(N_CORES)))
    global LAST_EXEC_WALL_NS
    LAST_EXEC_WALL_NS = int((_time.perf_counter() - _t0) * 1e9)
    out_a = np.concatenate([res.results[c]["out_a_part"] for c in range(N_CORES)])
    out_b = np.concatenate([res.results[c]["out_b_part"] for c in range(N_CORES)])
    return out_a, out_b
